# revision 67
# baseline (speedup 1.0000x reference)
"""Tensor-parallel Llama sparse attention (tree-draft + paged KV prefix) on 8 TRN2 cores.

Sharding: core c owns kv-head c (K/V cache slice), its 4 query heads (Wq cols),
Wk/Wv cols, and the matching Wo rows. Each core computes a full [512, 4096]
partial output; the host sums the 8 partials.

Max-free softmax identity: with no max subtraction, lse = log(denom), so the
sigmoid-lse merge of the two branches collapses to
(O_prefix + O_cur) / (den_prefix + den_cur). Scores are tiny (|s| << 1), so
exp never overflows.

v3 (fp8 + rope-fold): hidden/WQKV stored fp8e4m3 (scaled x32/x64),
projected with DoubleRow matmuls (2x PE). RoPE's rotate_half is folded into
the score matmuls via s = (q*cos)@K + (q*sin)@(-R K): the host stacks
[K; -R K] per chunk and a DoubleRow fp8 matmul contracts both terms in one
pass (256-deep), so on-chip rope is just two elementwise multiplies per
q-block straight into the fp8 qc/qs tile. Cache tails beyond cache_len are
zeroed on the host so exp()=1 there and an exact integer correction is
subtracted from the softmax denominator. The Wo projection is interleaved
into the attention batch loop as PSUM-bank-sized quarters that fill PE
bubbles while the exp runs on ACT; output partials are written as fp16.

HW legality notes (BIR verifier): GPSIMD (Pool) must never touch PSUM;
tensor-tensor ops need equal base partitions when both inputs are SBUF
(hence the half-swapped sin_k table); DMA cannot read PSUM; DoubleRow
matmul outputs must sit at partition base 0; matmul out base must be in
{0,32,64}. PSUM accumulation groups may share a bank only with disjoint
partition ranges.
"""
import math
import sys

import ml_dtypes
import numpy as np

sys.path.insert(0, "/opt/trn_rl_repo")

B, Q, H = 8, 64, 4096
NH, NKV, HD, G = 32, 8, 128, 4
L, M = 4096, 512

SCALE_HS = 32.0
SCALE_W = 64.0
DESCALE = 1.0 / (SCALE_HS * SCALE_W)
QSC = 256.0
# q/k tables are stored unscaled; exp() folds the fp8-projection descale and
# the 1/sqrt(d) score scale (tree side gets the k-projection descale twice)
ESC_P = DESCALE / math.sqrt(128.0)
ESC_T = DESCALE * DESCALE / math.sqrt(128.0)

LAST_EXEC_NS = None
LAST_RESULTS = None

# fp8 table blob column layout
C_COSQ, C_SINQ, C_COSK, C_SINK = 0, 512, 1024, 1536
C_SINK2 = 2048          # sin_k with partition halves swapped
NB8 = 2560


def _build_program(nls, nmask):
    import concourse.mybir as mybir
    from concourse import bacc, tile

    F32 = mybir.dt.float32
    F16 = mybir.dt.float16
    BF16 = mybir.dt.bfloat16
    FP8 = mybir.dt.float8e4
    EXP = mybir.ActivationFunctionType.Exp
    COPY = mybir.ActivationFunctionType.Copy
    DR = mybir.MatmulPerfMode.DoubleRow

    nc = bacc.Bacc("TRN2", target_bir_lowering=False, debug=False, num_devices=8)

    hsw = nc.dram_tensor("hsw", [128, 32, 1280], FP8, kind="ExternalInput").ap()
    wo = nc.dram_tensor("wo", [512, H], BF16, kind="ExternalInput").ap()
    k_t = nc.dram_tensor("k_t", [B, HD, 2 * L], FP8, kind="ExternalInput").ap()
    v = nc.dram_tensor("v", [B, HD, L], BF16, kind="ExternalInput").ap()
    blob8 = nc.dram_tensor("blob8", [128, NB8], FP8, kind="ExternalInput").ap()
    ones16 = nc.dram_tensor("ones16", [128, 1], BF16, kind="ExternalInput").ap()
    blob32 = nc.dram_tensor("blob32", [128, 128], F32, kind="ExternalInput").ap()
    m01 = nc.dram_tensor("m01", [64, B * 256], BF16, kind="ExternalInput").ap()
    out = nc.dram_tensor("out", [M, H], F16, kind="ExternalOutput").ap()

    with tile.TileContext(nc) as tc, nc.allow_low_precision("fp8 kernel"):
        with tc.tile_pool(name="const", bufs=1) as const:
            blob8_sb = const.tile([128, NB8], FP8, tag="blob8")
            ones_t = const.tile([128, 1], BF16, tag="ones16")
            blob32_sb = const.tile([128, 128], F32, tag="blob32")
            m01_sb = const.tile([64, B * 256], BF16, tag="m01")
            qcs = const.tile([HD, 4096], FP8, tag="qcs")   # (two, b, g, q)
            ktx = const.tile([HD, 1024], FP8, tag="ktx")   # (two, b*q)
            vnew = [const.tile([64, HD], BF16, tag=f"vn{t}", name=f"vn{t}")
                    for t in range(8)]
            attn_t = const.tile([HD, 2048], BF16, tag="attn")    # (g, b, q)
            wos = [const.tile([HD, H], BF16, tag=f"wo{g}", name=f"wo{g}")
                   for g in range(G)]

            warm = const.tile([1, 2], F32, tag="warm")
            nc.vector.memset(warm[:, 0:1], 0.0)
            nc.scalar.activation(warm[:, 1:2], warm[:, 0:1], EXP)

            cosq = blob8_sb[:, C_COSQ:C_COSQ + 512]
            sinq = blob8_sb[:, C_SINQ:C_SINQ + 512]
            cosk = blob8_sb[:, C_COSK:C_COSK + 512]
            sink = blob8_sb[:, C_SINK:C_SINK + 512]
            sink2 = blob8_sb[:, C_SINK2:C_SINK2 + 512]
            ones_sb = ones_t[:]

            # ---------------- QKV^T projection (fp8 DoubleRow) ----------------
            kvstack = tc.tile_pool(name="ktp", bufs=3)
            ktp = kvstack.__enter__()
            kvstack2 = tc.tile_pool(name="vip", bufs=3)
            vip = kvstack2.__enter__()
            kv_cache = {}

            def load_kv(b):
                nl = nls[b]
                kb = ktp.tile([HD, 2 * L], FP8, tag="kb", name=f"kb{b}")
                nc.sync.dma_start(kb[:, :nl * 256], k_t[b, :, :nl * 256])
                vb_t = vip.tile([HD, L], BF16, tag="vb", name=f"vb{b}")
                nc.sync.dma_start(vb_t[:, :nl * 128], v[b, :, :nl * 128])
                kv_cache[b] = (kb, vb_t)

            qcs_v = qcs[:].rearrange("p (w b g q) -> p w b g q",
                                     w=2, b=B, g=G, q=Q)
            ktx_v = ktx[:].rearrange("p (w t) -> p w t", w=2)
            hsp_stack = tc.tile_pool(name="hsp", bufs=1)
            hsp = hsp_stack.__enter__()
            with tc.tile_pool(name="qkv_ps", bufs=1, space="PSUM") as qkv_ps, \
                 tc.tile_pool(name="rope", bufs=1) as rope:
                hq = hsp.tile([128, 32 * 1280], FP8, tag="hsqkv")
                hq_v = hq[:].rearrange("p (c e) -> p c e", c=32, e=1280)
                # ramp piece sizes so the PE starts early and isn't left a
                # big backlog after the last piece; rope tables mid-stream
                splits = [(0, 2), (2, 5), (5, 11), (11, 18), (18, 25),
                          (25, 30), (30, 32)]
                for i, (c0, c1) in enumerate(splits):
                    nc.sync.dma_start(hq_v[:, c0:c1, :], hsw[:, c0:c1, :])
                    if i == 2:
                        nc.sync.dma_start(blob8_sb[:], blob8)
                nc.sync.dma_start(blob32_sb[:], blob32)
                nc.sync.dma_start(m01_sb[:], m01)
                nc.sync.dma_start(ones_t[:], ones16)
                qk_psum = [qkv_ps.tile([HD, M], F32, tag=f"qkv{m}", name=f"qkv{m}")
                           for m in range(6)]
                # k-outer so each superchunk's 6 matmuls run as its DMA lands
                # (m-outer would head-of-line block the PE on the last chunk)
                for k in range(16):
                    rhs = hq_v[:, 2 * k:2 * k + 2, 0:512]
                    for m in range(6):
                        lhsT = hq_v[:, 2 * k:2 * k + 2,
                                    512 + m * 128:512 + (m + 1) * 128]
                        nc.tensor.matmul(qk_psum[m][:], lhsT, rhs,
                                         start=(k == 0), stop=(k == 15),
                                         perf_mode=DR, skip_group_check=True)
                # prefetch K/V for first batches while RoPE runs
                load_kv(0)
                load_kv(1)

                # ---------------- RoPE ----------------
                # Fold rotate_half into the score matmuls:
                #   s = (q*cos)@K + (q*sin)@(-R K)
                # with [K; -R K] stacked host-side and contracted by a
                # DoubleRow fp8 matmul. On-chip rope for q is just two
                # elementwise multiplies straight into the fp8 qcs tile.
                vt_sb = rope.tile([HD, M], F32, tag="vt")
                nc.scalar.activation(vt_sb[:], qk_psum[5][:], COPY,
                                     scale=DESCALE)
                raws = {}
                for j in (0, 1, 2, 4):
                    raw = rope.tile([HD, M], BF16, tag=f"raw{j}",
                                    name=f"raw{j}")
                    nc.scalar.copy(raw[:], qk_psum[j][:])
                    raws[j] = raw
                cos3 = cosq.rearrange("p (b q) -> p b q", b=B)
                sin3 = sinq.rearrange("p (b q) -> p b q", b=B)
                # batches 0-3 first (gates b0's scores), then 4-7 which can
                # overlap the start of attention
                for half in (slice(0, 4), slice(4, 8)):
                    for j in range(4):
                        if j < 3:
                            # gpsimd cannot touch PSUM: multiply the
                            # ACT-evicted bf16 copy instead
                            r3 = raws[j][:].rearrange("p (b q) -> p b q", b=B)
                            nc.gpsimd.tensor_mul(qcs_v[:, 0, half, j, :],
                                                 r3[:, half, :],
                                                 cos3[:, half, :])
                            nc.gpsimd.tensor_mul(qcs_v[:, 1, half, j, :],
                                                 r3[:, half, :],
                                                 sin3[:, half, :])
                        else:
                            ps3 = qk_psum[j][:].rearrange(
                                "p (b q) -> p b q", b=B)
                            nc.vector.tensor_mul(qcs_v[:, 0, half, j, :],
                                                 ps3[:, half, :],
                                                 cos3[:, half, :])
                            nc.vector.tensor_mul(qcs_v[:, 1, half, j, :],
                                                 ps3[:, half, :],
                                                 sin3[:, half, :])
                # tree k: classic rope (signed sin_k table, cross-partition
                # halves) on the SBUF copy, then the stacked [-R k_roped] copy
                with tc.tile_pool(name="rope2", bufs=1) as rope2:
                    tcs = rope.tile([HD, M], BF16, tag="tcs_k")
                    tsn = rope2.tile([HD, M], BF16, tag="tsn_k")
                    raw4 = raws[4]
                    nc.gpsimd.tensor_mul(tcs[:], raw4[:], cosk)
                    nc.vector.tensor_mul(tsn[0:64, :], raw4[64:128, :],
                                         sink2[64:128, :])
                    nc.vector.tensor_mul(tsn[64:128, :], raw4[0:64, :],
                                         sink2[0:64, :])
                    nc.vector.tensor_add(ktx_v[:, 0, :], tcs[:], tsn[:])
                    nc.gpsimd.tensor_copy(ktx_v[0:64, 1, :],
                                          ktx_v[64:128, 0, :])
                    nc.gpsimd.tensor_scalar_mul(ktx_v[64:128, 1, :],
                                                ktx_v[0:64, 0, :], -1.0)

            hsp_stack.__exit__(None, None, None)

            # ------- attention per batch, Wo projection interleaved -------
            at_v = attn_t[:].rearrange("p (g b q) -> p g b q", g=G, b=B)
            with tc.tile_pool(name="ppool", bufs=4) as ppool, \
                 tc.tile_pool(name="pspool", bufs=2) as pspool, \
                 tc.tile_pool(name="small", bufs=2) as small, \
                 tc.tile_pool(name="oev", bufs=3) as oev, \
                 tc.tile_pool(name="mm_ps", bufs=2, space="PSUM") as mm_ps, \
                 tc.tile_pool(name="wo_ps", bufs=1, space="PSUM") as wo_ps, \
                 tc.tile_pool(name="o_ps", bufs=2, space="PSUM") as o_ps, \
                 tc.tile_pool(name="ds_ps", bufs=1, space="PSUM") as ds_ps:

                def emit_merge(st):
                    b, o_tile, ds = st
                    o_acc = o_tile[:]
                    den = ds[64:65, 256:512]
                    dsub = small.tile([1, 256], F32, tag="dsub",
                                      name=f"dsub{b}")
                    nc.vector.tensor_scalar_add(dsub[:], den,
                                                float(-nmask[b]))
                    recip = small.tile([1, 256], F32, tag="recip",
                                       name=f"recip{b}")
                    nc.vector.reciprocal(recip[:], dsub[:])
                    bc = small.tile([HD, 256], F32, tag="bc", name=f"bc{b}")
                    nc.gpsimd.partition_broadcast(bc[:], recip[:])
                    nc.vector.tensor_mul(
                        at_v[:, :, b, :],
                        o_acc.rearrange("p (g q) -> p g q", g=G),
                        bc[:].rearrange("p (g q) -> p g q", g=G),
                    )

                # Wo blocks: one quarter per (mt, nb, nn); block (mt, *)
                # needs attn_t for batches {2mt, 2mt+1} (after merge(2mt+1))
                def emit_wo_quarter(mt, nb, nn, pool=None):
                    if pool is None:
                        wp = wo_ps.tile([HD, 512], F32, tag="wp",
                                        name=f"wp{mt}_{nb}_{nn}")
                    else:
                        wp = pool.tile([HD, 1024], F32, tag="sc",
                                       name=f"wp{mt}_{nb}_{nn}")[:, 0:512]
                    c0 = nb * 2048 + nn * 512
                    for g in range(G):
                        lhsT = attn_t[:, g * 512 + mt * 128:
                                      g * 512 + (mt + 1) * 128]
                        nc.tensor.matmul(
                            wp, lhsT, wos[g][:, c0:c0 + 512],
                            start=(g == 0), stop=(g == 3),
                            skip_group_check=True)
                    ev = oev.tile([HD, 512], F16, tag="ev",
                                  name=f"ev{mt}_{nb}_{nn}")
                    if (mt * 8 + nb * 4 + nn) % 4 == 3:
                        nc.scalar.activation(ev[:], wp, COPY)
                    else:
                        nc.vector.tensor_copy(ev[:], wp)
                    nc.sync.dma_start(
                        out[mt * 128:(mt + 1) * 128, c0:c0 + 512], ev[:])

                wo_ready = []   # emission-ready (mt, nb, nn) triples
                pending = None
                qcount = 0
                for b in range(B):
                    nl = nls[b]
                    kb, vb_t = kv_cache.pop(b)
                    if b + 2 < B:
                        load_kv(b + 2)
                    if b < 2:
                        for g in (2 * b, 2 * b + 1):
                            nc.sync.dma_start(wos[g][:],
                                              wo[g * 128:(g + 1) * 128, :])
                    qrhs = qcs_v[:, :, b, :, :]
                    # o_acc in its own bank; den (partition 0) and the
                    # tree scores (partitions 64:128) share a second bank --
                    # PSUM accumulation groups may share a bank only with
                    # disjoint partition ranges (has_written is marked
                    # bank-wide for the writer's partitions)
                    o_tile = o_ps.tile([HD, 256], F32, tag="oacc",
                                       name=f"oacc{b}")
                    ds = ds_ps.tile([HD, 512], F32, tag="ds", name=f"ds{b}")
                    o_acc = o_tile[:]
                    den = ds[64:65, 256:512]
                    s2 = ds[0:64, 256:512]

                    nq = (nl + 3) // 4
                    quads = []  # (pt_tile, chunk_list)
                    den_q = []
                    for qi in range(nq + 1):
                        if qi < nq:
                            chunks = list(range(qi * 4, min(nl, qi * 4 + 4)))
                            cc = len(chunks)
                            sc = mm_ps.tile([HD, 1024], F32, tag="sc",
                                            name=f"sc{b}_{qi}")
                            for i, c in enumerate(chunks):
                                kbx = kb[:, c * 256:(c + 1) * 256].rearrange(
                                    "p (w k) -> p w k", w=2)
                                nc.tensor.matmul(
                                    sc[:, i * 256:(i + 1) * 256], kbx, qrhs,
                                    start=True, stop=True, perf_mode=DR)
                            pt = ppool.tile([HD, 1024], BF16, tag="pt",
                                            name=f"pt{b}_{qi}")
                            h1 = min(cc, 1) * 256
                            nc.scalar.activation(pt[:, :h1], sc[:, :h1], EXP,
                                                 scale=ESC_P)
                            if cc > 1:
                                nc.scalar.activation(pt[:, h1:cc * 256],
                                                     sc[:, h1:cc * 256], EXP,
                                                     scale=ESC_P)
                            quads.append((pt, chunks))
                        if b == 0 and qi == 2:
                            # V transposes for the tree branch, after b0's
                            # first scores so they don't head-block the PE;
                            # borrow the (still idle) wo psum slot
                            for t_ in range(4):
                                tp = wo_ps.tile([HD, 512], F32, tag="wp",
                                                name=f"tp{t_}")[:, 0:128]
                                nc.tensor.transpose(
                                    tp, vt_sb[:, t_ * 128:(t_ + 1) * 128],
                                    blob32_sb[:])
                                nc.vector.tensor_copy(vnew[2 * t_][:],
                                                      tp[0:64, :])
                                nc.vector.tensor_copy(vnew[2 * t_ + 1][:],
                                                      tp[64:128, :])
                        if qi == 2 or (qi == 1 and nq == 2):
                            # tree scores (needed only at the batch close)
                            nc.tensor.matmul(
                                s2, ktx_v[:, :, b * 64:(b + 1) * 64], qrhs,
                                start=True, stop=True, perf_mode=DR)
                            p2 = small.tile([Q, 256], BF16, tag="p2",
                                            name=f"p2_{b}")
                            nc.scalar.activation(p2[:], s2, EXP,
                                                 scale=ESC_T)
                            p2m = small.tile([Q, 256], BF16, tag="p2m",
                                             name=f"p2m_{b}")
                            nc.gpsimd.tensor_mul(
                                p2m[:], p2[:],
                                m01_sb[:, b * 256:(b + 1) * 256])
                        if qi == 2 and pending is not None:
                            emit_merge(pending)
                            pb = pending[0]
                            if pb % 2 == 1:
                                for nn_ in range(4):
                                    for nb_ in range(2):
                                        wo_ready.append((pb // 2, nb_, nn_))
                            pending = None
                        # Wo filler between a quad's scores and its PV (the
                        # PE would otherwise idle waiting for the exp)
                        qcount += 1
                        if wo_ready and (b >= 3 or qi >= 3):
                            emit_wo_quarter(*wo_ready.pop(0))
                            if len(wo_ready) > 6:
                                emit_wo_quarter(*wo_ready.pop(0))
                        if qi > 0:
                            pt, chunks = quads[qi - 1]
                            cc = len(chunks)
                            for i, c in enumerate(chunks):
                                nc.tensor.matmul(
                                    o_acc, vb_t[:, c * 128:(c + 1) * 128],
                                    pt[:, i * 256:(i + 1) * 256],
                                    start=(c == 0), stop=False,
                                    skip_group_check=True)
                            # partition pre-sum on DVE, then one den matmul
                            if cc == 4:
                                ps1 = pspool.tile([HD, 512], BF16, tag="ps1",
                                                  name=f"ps1_{b}_{qi}")
                                nc.gpsimd.tensor_add(ps1[:], pt[:, 0:512],
                                                     pt[:, 512:1024])
                                ps2 = pspool.tile([HD, 256], BF16, tag="ps2",
                                                  name=f"ps2_{b}_{qi}")
                                nc.gpsimd.tensor_add(ps2[:], ps1[:, 0:256],
                                                     ps1[:, 256:512])
                                drhs = ps2[:]
                            elif cc == 3:
                                ps1 = pspool.tile([HD, 512], BF16, tag="ps1",
                                                  name=f"ps1_{b}_{qi}")
                                nc.gpsimd.tensor_add(ps1[:, 0:256],
                                                     pt[:, 0:256],
                                                     pt[:, 256:512])
                                ps2 = pspool.tile([HD, 256], BF16, tag="ps2",
                                                  name=f"ps2_{b}_{qi}")
                                nc.gpsimd.tensor_add(ps2[:], ps1[:, 0:256],
                                                     pt[:, 512:768])
                                drhs = ps2[:]
                            elif cc == 2:
                                ps2 = pspool.tile([HD, 256], BF16, tag="ps2",
                                                  name=f"ps2_{b}_{qi}")
                                nc.gpsimd.tensor_add(ps2[:], pt[:, 0:256],
                                                     pt[:, 256:512])
                                drhs = ps2[:]
                            else:
                                drhs = pt[:, 0:256]
                            den_q.append(drhs)
                            if qi >= 2:
                                nc.tensor.matmul(den, ones_sb, den_q[qi - 2],
                                                 start=(qi == 2), stop=False,
                                                 skip_group_check=True)
                    # close with current-token tree attention
                    nc.tensor.matmul(o_acc, vnew[b][:], p2m[:],
                                     start=False, stop=True,
                                     skip_group_check=True)
                    nc.tensor.matmul(den, ones_sb, den_q[nq - 1],
                                     start=False, stop=False,
                                     skip_group_check=True)
                    nc.tensor.matmul(den, ones_sb[0:Q, :], p2m[:],
                                     start=False, stop=True,
                                     skip_group_check=True)
                    pending = (b, o_tile, ds)
                emit_merge(pending)
                for nn_ in range(4):
                    for nb_ in range(2):
                        wo_ready.append((3, nb_, nn_))
                ti = 0
                while wo_ready:
                    pool = [None, mm_ps, mm_ps][ti % 3]
                    emit_wo_quarter(*wo_ready.pop(0), pool=pool)
                    ti += 1

            kvstack2.__exit__(None, None, None)
            kvstack.__exit__(None, None, None)
    nc.compile()
    return nc


def prepare(hidden_states, Wq, Wk, Wv, Wo, K_cache, V_cache, cos, sin,
            tree_mask, position_ids, cache_lens):
    scale = 1.0 / math.sqrt(HD)
    f8 = ml_dtypes.float8_e4m3
    bf = ml_dtypes.bfloat16

    hs_t = np.ascontiguousarray(
        np.asarray(hidden_states, np.float32).reshape(M, H).T) * SCALE_HS

    cl = np.asarray(cache_lens, np.int32)
    nls = [max(1, int(math.ceil(int(c) / 128.0))) for c in cl]
    nmask = [nls[b] * 128 - int(cl[b]) for b in range(B)]

    pos = np.asarray(position_ids, np.int32)
    cosg = np.asarray(cos, np.float32)[pos].reshape(M, HD)
    sing = np.asarray(sin, np.float32)[pos].reshape(M, HD)
    sign = np.concatenate([-np.ones(64, np.float32), np.ones(64, np.float32)])
    cos_t = np.ascontiguousarray(cosg.T)
    sin_t = np.ascontiguousarray(sing.T)          # plain (q-side rope fold)
    sin_ts = sin_t * sign[:, None]                # signed (tree-k rope)
    fq = QSC * DESCALE * scale
    fk = DESCALE * 64.0

    blob8 = np.zeros((128, NB8), np.float32)
    blob8[:, C_COSQ:C_COSQ + 512] = cos_t
    blob8[:, C_SINQ:C_SINQ + 512] = sin_t
    blob8[:, C_COSK:C_COSK + 512] = cos_t
    blob8[:, C_SINK:C_SINK + 512] = sin_ts
    blob8[0:64, C_SINK2:C_SINK2 + 512] = sin_ts[64:128]
    blob8[64:128, C_SINK2:C_SINK2 + 512] = sin_ts[0:64]
    blob8 = blob8.astype(f8)
    ones16 = np.ones((128, 1), np.float32).astype(bf)

    blob32 = np.eye(HD, dtype=np.float32)

    tm = np.asarray(tree_mask, np.int32).astype(np.float32)
    m01 = np.ascontiguousarray(
        np.tile(tm.transpose(0, 2, 1), (1, 1, G)).transpose(1, 0, 2)
        .reshape(64, B * 256)).astype(bf)

    nc = _build_program(nls, nmask)

    Wq = np.asarray(Wq, np.float32) * SCALE_W
    Wk = np.asarray(Wk, np.float32) * SCALE_W
    Wv = np.asarray(Wv, np.float32) * SCALE_W
    Wo = np.asarray(Wo, np.float32)
    Kc = np.array(np.asarray(K_cache, np.float32), copy=True)
    Vc = np.array(np.asarray(V_cache, np.float32), copy=True)
    # zero the cache tail beyond cache_len: exp(score)=1 there, corrected by
    # subtracting nmask from the softmax denominator on-device
    for b in range(B):
        Kc[b, cl[b]:] = 0.0
        Vc[b, cl[b]:] = 0.0

    in_maps = []
    for c in range(8):
        # stacked [K; -R K]: block c -> [K_c(128 keys); K2_c; -K1_c]
        K_T = Kc[:, :, c, :].transpose(0, 2, 1)          # [B, 128, L]
        kx = np.zeros((B, HD, 32, 2, 128), np.float32)
        kx[:, :, :, 0, :] = K_T.reshape(B, HD, 32, 128)
        kx[:, 0:64, :, 1, :] = K_T[:, 64:128].reshape(B, 64, 32, 128)
        kx[:, 64:128, :, 1, :] = -K_T[:, 0:64].reshape(B, 64, 32, 128)
        kx = kx.reshape(B, HD, 2 * L)
        w_qkv = np.concatenate(
            [Wq[:, c * 512:(c + 1) * 512],
             Wk[:, c * 128:(c + 1) * 128],
             Wv[:, c * 128:(c + 1) * 128]], axis=1)  # [H, 768]
        hsw = np.zeros((128, 32, 1280), np.float32)
        for ch in range(32):
            hsw[:, ch, 0:512] = hs_t[ch * 128:(ch + 1) * 128, :]
            hsw[:, ch, 512:1280] = w_qkv[ch * 128:(ch + 1) * 128, :]
        in_maps.append(dict(
            hsw=hsw.astype(f8),
            wo=np.ascontiguousarray(Wo[c * 512:(c + 1) * 512, :]).astype(bf),
            k_t=kx.astype(f8),
            v=np.ascontiguousarray(
                Vc[:, :, c, :].reshape(B, 32, 128, HD).transpose(0, 2, 1, 3)
                .reshape(B, HD, L)).astype(bf),
            blob8=blob8, ones16=ones16, blob32=blob32, m01=m01,
        ))

    return nc, in_maps


def kernel(**inputs):
    global LAST_EXEC_NS, LAST_RESULTS
    from concourse.bass_utils import run_bass_kernel_spmd

    nc, in_maps = prepare(**inputs)
    res = run_bass_kernel_spmd(nc, in_maps, core_ids=list(range(8)))
    LAST_EXEC_NS = res.exec_time_ns
    LAST_RESULTS = res
    out = np.zeros((M, H), np.float32)
    for r_ in res.results:
        out += np.asarray(r_["out"], np.float32)
    return out.reshape(B, Q, H).astype(np.float32)
